# revision 4
# baseline (speedup 1.0000x reference)
"""BANLayer (bilinear attention) Trainium2 kernel.

Full inputs -> shard batch over 8 NeuronCores -> Bass/Tile kernel per core
-> gather -> host BatchNorm tail (needs cross-core batch statistics).

Per core (BPC=4 batches):
  vT/qT       : PE transposes of v/q tiles (contraction dim must be on
                partitions), drained PSUM->SBUF as float32r (TF32 path,
                1 cycle/row matmuls vs 4 for fp32).
  projections : vp_T [k,v], qp_T [k,q] (relu, per-partition bias),
                vp_nat [v,k] (for the fusion matmul's lhsT).
  att         : per head h: att[v,q] = sum_k (vp_T*h_mat[h])[k,v]^T qp_T[k,q],
                PSUM accumulated over 6 k-tiles; drained by ACT Exp with
                per-tile row-sum accumulators (softmax over the full V*Q
                map; max-subtraction skipped -- logits are O(1) by
                construction so exp cannot overflow).
  softmax     : Z via tensor_reduce + ones-matmul partition reduce,
                reciprocal, tensor_scalar row scale; att written to HBM.
  fusion      : A = att0+att1; D^T[k,q] = sum_v vp_nat[v,k]^T A[v,q];
                fusion[k] = sum_q D^T[k,q]*qp_T[k,q] via fused
                scalar_tensor_tensor accumulate. AvgPool*k + BatchNorm
                done on host (8K elements, needs all 32 batches).
"""

import numpy as np

import concourse.bacc as bacc
import concourse.mybir as mybir
import concourse.tile as tile
from concourse.bass_utils import run_bass_kernel_spmd

F32 = mybir.dt.float32
F32R = mybir.dt.float32r
AF = mybir.ActivationFunctionType
ALU = mybir.AluOpType
AX = mybir.AxisListType

NCORES = 8
B, V, Q, D = 32, 512, 1024, 128
HK, HOUT, KGRP = 768, 2, 3
HDIM = HK // KGRP  # 256
BPC = B // NCORES  # batches per core
KT = HK // 128     # 6 k-tiles
VT = V // 128      # 4 v-tiles
QT = Q // 128      # 8 q-tiles
EPS = 1e-5

_BUILD_CACHE = {}


def build(softmax_on, bv_zero, bq_zero, h_bias, repeat=1):
    nc = bacc.Bacc("TRN2", target_bir_lowering=False, debug=False)

    v_h = nc.dram_tensor("v", [BPC, V, D], F32, kind="ExternalInput")
    q_h = nc.dram_tensor("q", [BPC, Q, D], F32, kind="ExternalInput")
    wvt_h = nc.dram_tensor("wvt", [D, HK], F32, kind="ExternalInput")
    wqt_h = nc.dram_tensor("wqt", [D, HK], F32, kind="ExternalInput")
    hm_h = nc.dram_tensor("hm", [128, HOUT, KT], F32, kind="ExternalInput")
    bv_h = nc.dram_tensor("bvb", [128, KT], F32, kind="ExternalInput")
    bq_h = nc.dram_tensor("bqb", [128, KT], F32, kind="ExternalInput")
    id_h = nc.dram_tensor("ident", [128, 128], F32, kind="ExternalInput")
    att_h = nc.dram_tensor("att", [BPC, HOUT, V, Q], F32, kind="ExternalOutput")
    fus_h = nc.dram_tensor("fus", [BPC, 128, KT], F32, kind="ExternalOutput")

    with tile.TileContext(nc) as tc:
        with tc.tile_pool(name="pc", bufs=1) as pc, \
             tc.tile_pool(name="pin", bufs=2) as pin, \
             tc.tile_pool(name="ptr", bufs=2) as ptr, \
             tc.tile_pool(name="pproj", bufs=1) as pproj, \
             tc.tile_pool(name="pmap", bufs=1) as pmap, \
             tc.tile_pool(name="psc", bufs=2) as psc, \
             tc.tile_pool(name="pa", bufs=2, space="PSUM") as pa, \
             tc.tile_pool(name="pb", bufs=2, space="PSUM") as pb:

            # ---- constants ----
            wvt = pc.tile([D, HK], F32)
            wqt = pc.tile([D, HK], F32)
            hm = pc.tile([128, HOUT, KT], F32)
            bvb = pc.tile([128, KT], F32)
            bqb = pc.tile([128, KT], F32)
            idt = pc.tile([128, 128], F32)
            nc.sync.dma_start(wvt[:], wvt_h[:])
            nc.sync.dma_start(wqt[:], wqt_h[:])
            nc.sync.dma_start(hm[:], hm_h[:])
            nc.sync.dma_start(bvb[:], bv_h[:])
            nc.sync.dma_start(bqb[:], bq_h[:])
            nc.sync.dma_start(idt[:], id_h[:])
            wvt_r = pc.tile([D, HK], F32R)
            wqt_r = pc.tile([D, HK], F32R)
            ones_f = pc.tile([128, 128], F32)
            nc.vector.tensor_copy(wvt_r[:], wvt[:])
            nc.vector.tensor_copy(wqt_r[:], wqt[:])
            # all-ones (plain fp32 -- used by the tiny partition-sum matmuls,
            # which must not be fp32r: s3d3_mm_fp32r_restrictions rejects N=1)
            nc.vector.tensor_scalar(ones_f[:], idt[:], 0.0, 1.0, ALU.mult, ALU.add)

            for _rep in range(repeat):
                for b in range(BPC):
                    # ---- load + transpose v, q ----
                    vin = pin.tile([128, VT, 128], F32, tag="vin")
                    qin = pin.tile([128, QT, 128], F32, tag="qin")
                    nc.sync.dma_start(vin[:], v_h[b].rearrange("(t p) d -> p t d", p=128))
                    nc.sync.dma_start(qin[:], q_h[b].rearrange("(t p) d -> p t d", p=128))
                    vT = ptr.tile([128, V], F32R, tag="vT")
                    qT = ptr.tile([128, Q], F32R, tag="qT")
                    for t in range(VT):
                        pt = pb.tile([128, 1024], F32, tag="pb")
                        nc.tensor.transpose(pt[:, 0:128], vin[:, t, :], idt[:])
                        nc.vector.tensor_copy(vT[:, t * 128:(t + 1) * 128], pt[:, 0:128])
                    for t in range(QT):
                        pt = pb.tile([128, 1024], F32, tag="pb")
                        nc.tensor.transpose(pt[:, 0:128], qin[:, t, :], idt[:])
                        nc.vector.tensor_copy(qT[:, t * 128:(t + 1) * 128], pt[:, 0:128])

                    # ---- projections ----
                    vpT = pproj.tile([128, KT, V], F32R, tag="vpT")
                    sv0 = pproj.tile([128, KT, V], F32R, tag="sv0")
                    sv1 = pproj.tile([128, KT, V], F32R, tag="sv1")
                    qpT = pproj.tile([128, KT, Q], F32R, tag="qpT")
                    vpn = pproj.tile([128, VT, HK], F32R, tag="vpn")
                    for kt in range(KT):
                        ks = slice(kt * 128, (kt + 1) * 128)
                        pt = pb.tile([128, 1024], F32, tag="pb")
                        nc.tensor.matmul(pt[:, 0:V], wvt_r[:, ks], vT[:],
                                         start=True, stop=True)
                        nc.scalar.activation(vpT[:, kt, :], pt[:, 0:V], AF.Relu,
                                             bias=bvb[:, kt:kt + 1])
                        nc.vector.tensor_scalar_mul(sv0[:, kt, :],
                                                    vpT[:, kt, :].bitcast(F32),
                                                    hm[:, 0, kt:kt + 1])
                        nc.vector.tensor_scalar_mul(sv1[:, kt, :],
                                                    vpT[:, kt, :].bitcast(F32),
                                                    hm[:, 1, kt:kt + 1])
                        pt = pb.tile([128, 1024], F32, tag="pb")
                        nc.tensor.matmul(pt[:, 0:512], wqt_r[:, ks], qT[:, 0:512],
                                         start=True, stop=True)
                        nc.tensor.matmul(pt[:, 512:1024], wqt_r[:, ks], qT[:, 512:1024],
                                         start=True, stop=True)
                        nc.scalar.activation(qpT[:, kt, :], pt[:], AF.Relu,
                                             bias=bqb[:, kt:kt + 1])
                    if bv_zero:
                        for vt in range(VT):
                            vs = slice(vt * 128, (vt + 1) * 128)
                            pt = pb.tile([128, 1024], F32, tag="pb")
                            # bank-aligned output splits: a matmul's PSUM
                            # output must not straddle a 512-fp32 bank
                            nc.tensor.matmul(pt[:, 0:512], vT[:, vs], wvt_r[:, 0:512],
                                             start=True, stop=True)
                            nc.tensor.matmul(pt[:, 512:768], vT[:, vs], wvt_r[:, 512:768],
                                             start=True, stop=True)
                            nc.scalar.activation(vpn[:, vt, :], pt[:, 0:768], AF.Relu)
                    else:
                        # general path: vp_nat = transpose of vp_T tiles
                        for vt in range(VT):
                            for kt in range(KT):
                                pt = pb.tile([128, 1024], F32, tag="pb")
                                nc.tensor.transpose(
                                    pt[:, 0:128],
                                    vpT[:, kt, vt * 128:(vt + 1) * 128].bitcast(F32),
                                    idt[:])
                                nc.vector.tensor_copy(
                                    vpn[:, vt, kt * 128:(kt + 1) * 128], pt[:, 0:128])

                    # ---- attention + softmax per head ----
                    emaps = []
                    for h in range(HOUT):
                        sv = sv0 if h == 0 else sv1
                        e = pmap.tile([128, VT, Q], F32, tag=f"e{h}")
                        emaps.append(e)
                        zp = psc.tile([128, VT], F32, tag=f"zp{h}")
                        for vt in range(VT):
                            vs = slice(vt * 128, (vt + 1) * 128)
                            pt = pa.tile([128, 1024], F32, tag="pa")
                            for kt in range(KT):
                                nc.tensor.matmul(pt[:, 0:512], sv[:, kt, vs],
                                                 qpT[:, kt, 0:512],
                                                 start=(kt == 0), stop=(kt == KT - 1))
                                nc.tensor.matmul(pt[:, 512:1024], sv[:, kt, vs],
                                                 qpT[:, kt, 512:1024],
                                                 start=(kt == 0), stop=(kt == KT - 1))
                            if softmax_on:
                                nc.scalar.activation(e[:, vt, :], pt[:], AF.Exp,
                                                     accum_out=zp[:, vt:vt + 1])
                            else:
                                nc.scalar.activation(e[:, vt, :], pt[:], AF.Copy,
                                                     bias=float(h_bias[h]))
                        if softmax_on:
                            zrow = psc.tile([128, 1], F32, tag=f"zr{h}")
                            nc.vector.tensor_reduce(zrow[:], zp[:], axis=AX.X, op=ALU.add)
                            pz = pb.tile([128, 1024], F32, tag="pb")
                            nc.tensor.matmul(pz[:, 0:1], ones_f[:], zrow[:],
                                             start=True, stop=True)
                            sinv = psc.tile([128, 1], F32, tag=f"si{h}")
                            nc.vector.reciprocal(sinv[:], pz[:, 0:1])
                            for vt in range(VT):
                                nc.vector.tensor_scalar_mul(e[:, vt, :], e[:, vt, :],
                                                            sinv[:])
                        for vt in range(VT):
                            nc.sync.dma_start(
                                att_h[b, h, vt * 128:(vt + 1) * 128, :], e[:, vt, :])

                    # ---- fusion ----
                    amap = pmap.tile([128, VT, Q], F32R, tag="amap")
                    for vt in range(VT):
                        nc.vector.tensor_add(amap[:, vt, :], emaps[0][:, vt, :],
                                             emaps[1][:, vt, :])
                    fus = psc.tile([128, KT], F32, tag="fus")
                    for kt in range(KT):
                        ks = slice(kt * 128, (kt + 1) * 128)
                        pt = pb.tile([128, 1024], F32, tag="pb")
                        for vt in range(VT):
                            nc.tensor.matmul(pt[:, 0:512], vpn[:, vt, ks],
                                             amap[:, vt, 0:512],
                                             start=(vt == 0), stop=(vt == VT - 1))
                            nc.tensor.matmul(pt[:, 512:1024], vpn[:, vt, ks],
                                             amap[:, vt, 512:1024],
                                             start=(vt == 0), stop=(vt == VT - 1))
                        esc = psc.tile([128, Q], F32, tag="esc")
                        nc.vector.scalar_tensor_tensor(
                            out=esc[:], in0=pt[:], scalar=1.0,
                            in1=qpT[:, kt, :].bitcast(F32),
                            op0=ALU.mult, op1=ALU.mult,
                            accum_out=fus[:, kt:kt + 1])
                    nc.sync.dma_start(fus_h[b], fus[:])

    nc.finalize()
    return nc


def _get_nc(key):
    if key not in _BUILD_CACHE:
        _BUILD_CACHE[key] = build(*key[:-1], repeat=key[-1])
    return _BUILD_CACHE[key]


def _prep_consts(Wv, bv, Wq, bq, h_mat):
    wvt = np.ascontiguousarray(Wv.T, dtype=np.float32)          # [128, 768]
    wqt = np.ascontiguousarray(Wq.T, dtype=np.float32)          # [128, 768]
    hm = np.ascontiguousarray(
        h_mat.reshape(HOUT, KT, 128).transpose(2, 0, 1), dtype=np.float32)
    bvb = np.ascontiguousarray(bv.reshape(KT, 128).T, dtype=np.float32)
    bqb = np.ascontiguousarray(bq.reshape(KT, 128).T, dtype=np.float32)
    ident = np.eye(128, dtype=np.float32)
    return wvt, wqt, hm, bvb, bqb, ident


def run_device(inputs, repeat=1):
    v = np.ascontiguousarray(np.asarray(inputs["v"], dtype=np.float32))
    q = np.ascontiguousarray(np.asarray(inputs["q"], dtype=np.float32))
    Wv = np.asarray(inputs["Wv"], dtype=np.float32)
    bv = np.asarray(inputs["bv"], dtype=np.float32)
    Wq = np.asarray(inputs["Wq"], dtype=np.float32)
    bq = np.asarray(inputs["bq"], dtype=np.float32)
    h_mat = np.asarray(inputs["h_mat"], dtype=np.float32)
    h_bias = np.asarray(inputs["h_bias"], dtype=np.float32)
    softmax_on = bool(int(np.asarray(inputs["softmax"]).reshape(-1)[0]))

    key = (softmax_on, not np.any(bv), not np.any(bq),
           tuple(float(x) for x in h_bias), repeat)
    nc = _get_nc(key)

    wvt, wqt, hm, bvb, bqb, ident = _prep_consts(Wv, bv, Wq, bq, h_mat)
    in_maps = []
    for c in range(NCORES):
        in_maps.append({
            "v": v[c * BPC:(c + 1) * BPC],
            "q": q[c * BPC:(c + 1) * BPC],
            "wvt": wvt, "wqt": wqt, "hm": hm,
            "bvb": bvb, "bqb": bqb, "ident": ident,
        })
    res = run_bass_kernel_spmd(nc, in_maps, list(range(NCORES)))
    return res


def kernel(**inputs):
    res = run_device(inputs)
    att = np.concatenate([res.results[c]["att"] for c in range(NCORES)], axis=0)
    fus_raw = np.concatenate([res.results[c]["fus"] for c in range(NCORES)], axis=0)
    # fus_raw[b, p, t] holds fusion[b, t*128 + p]
    fusion = fus_raw.transpose(0, 2, 1).reshape(B, HK)
    logits = fusion.reshape(B, HDIM, KGRP).sum(axis=-1)
    gamma = np.asarray(inputs["gamma"], dtype=np.float32)
    beta = np.asarray(inputs["beta"], dtype=np.float32)
    mu = logits.mean(axis=0)
    var = logits.var(axis=0)
    logits = ((logits - mu) / np.sqrt(var + EPS) * gamma + beta).astype(np.float32)
    return logits, att
